# revision 1
# baseline (speedup 1.0000x reference)
"""BlendShapes model kernel for 8 Trainium2 NeuronCores.

Computation (reference):
    pose_repr = pose[:, 1:].reshape(B, 23, 9) - eye      # (B, J, 9)
    per-joint MLP 9 -> 18 -> 32 -> 8 (ReLU between)      # coff (B, J, 8)
    basis_full = basis[:, None] * mask[:, :, None, None]  # (V, J, 8, 3)
    res = einsum('bjk,vjkc->bvc', coff, basis_full)       # (B, V, 3)

Mapping:
  - Vertices sharded across 8 cores (V=6890 padded to 8*864=6912; VC3=2592
    output columns per core). Each core runs the full MLP with activations
    transposed ([features, batch]) so coff^T feeds the main matmul's
    stationary operand directly.
  - basis*mask (x 2^13, exact) is precomputed on the host as one fp16
    [184, VC3] tensor per core - no on-chip broadcast or multiply.
  - The identity subtraction (pose - eye) is folded into the L1 bias on the
    host: b1_eff = b1 - w1^T eye.
  - L3 matmuls write a stacked [128, 512] / [56, 512] PSUM tile directly
    (tile_position column offsets 32c), so one bias-add per half produces
    coffT_a / coffT_b - no SBUF->SBUF partition-merge DMAs.
  - Main matmul per b-tile is K-grouped: all 6 N-slices with coffT_a
    stationary, then all 6 with coffT_b accumulating (2 stationary switches
    per b-tile). 3 PSUM pair-tiles [128, 1024] rotate with bufs=3; each pair
    is evacuated (ACT/DVE alternating, x 2^-13 descale) and stored as soon
    as its accumulation completes, stores alternating across the two HWDGE
    rings (sync/scalar).
"""

import numpy as np

N_VERT, N_JOINT, BPJ, BATCH = 6890, 23, 8, 1024
VPAD = 6912  # 8 * 864
VC = VPAD // 8  # 864 vertices per core
VC3 = VC * 3  # 2592
K_ALL = N_JOINT * BPJ  # 184
NB = BATCH // 128  # 8 b-tiles

# Unified joint chunking for the MLP: 4 joints per chunk (3 in the tail).
CHUNKS = [(0, 4), (4, 8), (8, 12), (12, 16), (16, 20), (20, 23)]
NCH = len(CHUNKS)


def _offsets(mpj):
    offs, col = [], 0
    for js, je in CHUNKS:
        offs.append(col)
        col += (je - js) * mpj
    return offs, col


W1_OFF, W1_TOT = _offsets(18)  # 414
W2_OFF, W2_TOT = _offsets(32)  # 736
W3_OFF, W3_TOT = _offsets(8)   # 184
W2_OFF = [W1_TOT + o for o in W2_OFF]
W3_OFF = [W1_TOT + W2_TOT + o for o in W3_OFF]
W_COLS = W1_TOT + W2_TOT + W3_TOT  # 1334

# bias_all columns: [0:6] L1 bias (eye term folded in), [6:12] L2 bias,
# [12] L3 bias stacked for chunks 0-3 (128 rows), [13] for chunks 4-5 (56).
BIAS_COLS = 14
BSCALE = 8192.0  # 2**13, exact
DESCALE = 1.0 / 8192.0

# Main matmul N pairs: each pair = one [128, 1024] (2-bank) PSUM tile,
# covering columns [1024p, 1024p + w0 + w1) via two matmuls.
PAIR_W = [(512, 512), (512, 512), (512, 32)]  # covers 2592

_CACHED = {}


def _build_nc():
    import concourse.tile as tile
    from concourse import bacc, mybir
    from contextlib import ExitStack

    dt = mybir.dt
    f32, f16 = dt.float32, dt.float16
    AF = mybir.ActivationFunctionType
    ALU = mybir.AluOpType

    nc = bacc.Bacc(None, target_bir_lowering=False)

    # pose packed as [100, 3072]: chunk c at rows [64*(c%2), +9nj), columns
    # [1024*(c//2), +1024). One DMA; row bands at 0/64 because a matmul's
    # moving operand must start at partition 0, 32, or 64 (odd chunks' L1
    # weight blocks are packed at row 64 in w_all to match).
    pose_t = nc.dram_tensor("pose_t", [100, 3 * BATCH], f16, kind="ExternalInput")
    w_all = nc.dram_tensor("w_all", [128, W_COLS], f16, kind="ExternalInput")
    bias_all = nc.dram_tensor("bias_all", [128, BIAS_COLS], f32, kind="ExternalInput")
    bfm = nc.dram_tensor("bfm", [K_ALL, VC3], f16, kind="ExternalInput")
    res = nc.dram_tensor("res", [BATCH, VC3], f32, kind="ExternalOutput")

    with ExitStack() as ctx:
        tc = ctx.enter_context(tile.TileContext(nc))
        const = ctx.enter_context(tc.tile_pool(name="const", bufs=1))
        work = ctx.enter_context(tc.tile_pool(name="work", bufs=1))
        outp = ctx.enter_context(tc.tile_pool(name="outp", bufs=3))
        pmlp = ctx.enter_context(tc.tile_pool(name="pmlp", bufs=2, space="PSUM"))
        pmain = ctx.enter_context(tc.tile_pool(name="pmain", bufs=3, space="PSUM"))

        # ---- input DMAs, finest-first across both HWDGE rings so the L1
        # critical path (w1 block + pose block 0) lands in ~2 small DMAs;
        # later-needed data (L2/L3 weights, pose blocks 1-2, bfm) follows.
        pose_sb = work.tile([100, 3 * BATCH], f16, tag="pose")
        w_sb = const.tile([128, W_COLS], f16, tag="w")
        bias_sb = const.tile([128, BIAS_COLS], f32, tag="bias")
        # pose blocks 0-1 (chunks 0-3) land together as one sync DMA while
        # the L1 weight block arrives on the scalar ring; pose block 2 and
        # the L2/L3 weights follow, then bfm.
        nc.sync.dma_start(
            out=pose_sb[:, 0 : 2 * BATCH], in_=pose_t[:, 0 : 2 * BATCH]
        )
        nc.scalar.dma_start(out=w_sb[:, 0:W1_TOT], in_=w_all[:, 0:W1_TOT])
        nc.sync.dma_start(out=bias_sb[:], in_=bias_all[:, :])
        nc.scalar.dma_start(
            out=pose_sb[:, 2 * BATCH : 3 * BATCH], in_=pose_t[:, 2 * BATCH : 3 * BATCH]
        )
        nc.scalar.dma_start(out=w_sb[:, W1_TOT:], in_=w_all[:, W1_TOT:])

        def pose_ap(c, hs):
            K = 9 * (CHUNKS[c][1] - CHUNKS[c][0])
            r0 = 64 * (c % 2)
            c0 = BATCH * (c // 2)
            return pose_sb[r0 : r0 + K, c0 + hs.start : c0 + hs.stop]

        bfm_a = work.tile([128, VC3], f16, tag="bfm_a")
        bfm_b = work.tile([56, VC3], f16, tag="bfm_b")
        nc.scalar.dma_start(out=bfm_a[:], in_=bfm[0:128, :])
        nc.scalar.dma_start(out=bfm_b[:], in_=bfm[128:K_ALL, :])

        coffT_a = work.tile([128, BATCH], f16, tag="coffT_a")
        coffT_b = work.tile([56, BATCH], f16, tag="coffT_b")
        h1 = {}
        h2 = {}

        def mlp_half(h):
            hs = slice(h * 512, (h + 1) * 512)
            # L1: 9nj -> 18nj, ReLU(x + b_eff). Chunks 2-4 borrow PSUM from
            # the pmain pool (idle during the MLP phase) so all six L1
            # matmuls can issue without waiting on epilogue recycling; the 3
            # borrowed allocations keep pmain's bufs=3 rotation aligned for
            # the following b-tiles.
            for c, (js, je) in enumerate(CHUNKS):
                nj = je - js
                K, M = 9 * nj, 18 * nj
                off = W1_OFF[c]
                r0 = 64 * (c % 2)
                if c in (2, 3, 4):
                    ps = pmain.tile([M, 512], f32, tag="pmain", name=f"ps1_{c}_{h}")
                else:
                    ps = pmlp.tile([M, 512], f32, tag="psmlp", name=f"ps1_{c}_{h}")
                nc.tensor.matmul(
                    ps[:], lhsT=w_sb[r0 : r0 + K, off : off + M], rhs=pose_ap(c, hs),
                    start=True, stop=True,
                )
                if h == 0:
                    h1[c] = work.tile([M, BATCH], f16, tag=f"h1_{c}", name=f"h1_{c}")
                if c % 2 == 0:
                    nc.scalar.activation(
                        h1[c][:, hs], ps[:], AF.Relu, bias=bias_sb[0:M, c : c + 1]
                    )
                else:
                    nc.vector.tensor_scalar(
                        out=h1[c][:, hs], in0=ps[:],
                        scalar1=bias_sb[0:M, c : c + 1], scalar2=0.0,
                        op0=ALU.add, op1=ALU.max,
                    )
            # L2: 18nj -> 32nj, ReLU; epilogues split ACT/DVE
            for c, (js, je) in enumerate(CHUNKS):
                nj = je - js
                K, M = 18 * nj, 32 * nj
                off = W2_OFF[c]
                ps = pmlp.tile([M, 512], f32, tag="psmlp", name=f"ps2_{c}_{h}")
                nc.tensor.matmul(
                    ps[:], lhsT=w_sb[0:K, off : off + M], rhs=h1[c][:, hs],
                    start=True, stop=True,
                )
                if h == 0:
                    h2[c] = work.tile([M, BATCH], f16, tag=f"h2_{c}", name=f"h2_{c}")
                if c % 2 == 0:
                    nc.scalar.activation(
                        h2[c][:, hs], ps[:], AF.Relu, bias=bias_sb[0:M, 6 + c : 7 + c]
                    )
                else:
                    nc.vector.tensor_scalar(
                        out=h2[c][:, hs], in0=ps[:],
                        scalar1=bias_sb[0:M, 6 + c : 7 + c], scalar2=0.0,
                        op0=ALU.add, op1=ALU.max,
                    )
            # L3: 32nj -> 8nj into stacked PSUM tiles (chunk c at partition
            # 32c), one bias-add each -> coffT_a / coffT_b.
            ps3a = pmlp.tile([128, 512], f32, tag="psmlp", name=f"ps3a_{h}")
            ps3b = pmlp.tile([56, 512], f32, tag="psmlp", name=f"ps3b_{h}")
            for c, (js, je) in enumerate(CHUNKS):
                nj = je - js
                K, M = 32 * nj, 8 * nj
                off = W3_OFF[c]
                if c < 4:
                    r0 = 32 * c
                    dst = ps3a[r0 : r0 + M, :]
                else:
                    r0 = 32 * (c - 4)
                    dst = ps3b[r0 : r0 + M, :]
                nc.tensor.matmul(
                    dst, lhsT=w_sb[0:K, off : off + M], rhs=h2[c][:, hs],
                    start=True, stop=True, tile_position=(0, r0),
                )
            nc.vector.tensor_scalar(
                out=coffT_a[:, hs], in0=ps3a[:], scalar1=bias_sb[0:128, 12:13],
                scalar2=None, op0=ALU.add,
            )
            nc.vector.tensor_scalar(
                out=coffT_b[:, hs], in0=ps3b[:], scalar1=bias_sb[0:56, 13:14],
                scalar2=None, op0=ALU.add,
            )

        def main_btile(bt):
            bsl = slice(bt * 128, (bt + 1) * 128)
            ostrip = outp.tile([128, VC3], f32, tag="ostrip", name=f"ostrip_{bt}")
            ps = [
                pmain.tile([128, 1024], f32, tag="pmain", name=f"ps_{bt}_{p}")
                for p in range(3)
            ]
            # K pass a (coffT_a stationary for all 6 N-slices; N=512 is the
            # ISA max), then pass b accumulating; pair p evacuates + stores
            # right after its b matmuls. All evacuations on DVE - ACT (MLP
            # epilogues + ring issues) stalled the pipeline when it carried
            # half of them.
            for p, (w0, w1) in enumerate(PAIR_W):
                c0 = 1024 * p
                nc.tensor.matmul(
                    ps[p][:, 0:w0], lhsT=coffT_a[:, bsl],
                    rhs=bfm_a[:, c0 : c0 + w0], start=True, stop=False,
                )
                nc.tensor.matmul(
                    ps[p][:, 512 : 512 + w1], lhsT=coffT_a[:, bsl],
                    rhs=bfm_a[:, c0 + 512 : c0 + 512 + w1], start=True, stop=False,
                )
            for p, (w0, w1) in enumerate(PAIR_W):
                c0 = 1024 * p
                nc.tensor.matmul(
                    ps[p][:, 0:w0], lhsT=coffT_b[:, bsl],
                    rhs=bfm_b[:, c0 : c0 + w0], start=False, stop=True,
                )
                nc.tensor.matmul(
                    ps[p][:, 512 : 512 + w1], lhsT=coffT_b[:, bsl],
                    rhs=bfm_b[:, c0 + 512 : c0 + 512 + w1], start=False, stop=True,
                )
                # bt3's evacuations go to ACT so MLP half 1 (right after
                # bt3) isn't gated on the DVE evac backlog; all other
                # b-tiles evacuate on DVE. The last b-tile evacuates and
                # stores in 512-column pieces so the final store (the 32-col
                # tail, 16 KB) completes almost immediately after the last
                # matmul.
                pieces = (
                    [(0, w0), (w0, w1)] if bt == NB - 1 else [(0, w0 + w1)]
                )
                for q, (o0, wq) in enumerate(pieces):
                    osl = slice(c0 + o0, c0 + o0 + wq)
                    if bt == 3:
                        nc.scalar.activation(
                            ostrip[:, osl], ps[p][:, o0 : o0 + wq], AF.Copy,
                            scale=DESCALE,
                        )
                    else:
                        nc.vector.tensor_scalar(
                            out=ostrip[:, osl], in0=ps[p][:, o0 : o0 + wq],
                            scalar1=DESCALE, scalar2=None, op0=ALU.mult,
                        )
                    ring = nc.sync if (3 * bt + p + q) % 2 == 0 else nc.scalar
                    ring.dma_start(out=res[bsl, osl], in_=ostrip[:, osl])

        mlp_half(0)
        for bt in range(4):
            main_btile(bt)
        mlp_half(1)
        for bt in range(4, NB):
            main_btile(bt)

    nc.finalize()
    return nc


def _pack_host(pose, basis, mask, w1, b1, w2, b2, w3, b3):
    pose_flat = pose[:, 1:].reshape(BATCH, 207).T.astype(np.float16)  # [207, B]
    pose_t = np.zeros((100, 3 * BATCH), np.float16)
    for c, (js, je) in enumerate(CHUNKS):
        K = 9 * (je - js)
        r0, c0 = 64 * (c % 2), BATCH * (c // 2)
        pose_t[r0 : r0 + K, c0 : c0 + BATCH] = pose_flat[9 * js : 9 * js + K]

    # bfm[8j+k, (v, c)] = mask[v, j] * basis[v, k, c] * 2^13, fp16
    basis_t = basis.transpose(1, 0, 2).reshape(BPJ, N_VERT * 3)  # [k, v3]
    mask3 = np.repeat(mask.T, 3, axis=1)  # [j, v3]
    bfm = np.zeros((K_ALL, VPAD * 3), np.float16)
    bfm[:, : N_VERT * 3] = (
        mask3[:, None, :] * (basis_t * BSCALE)[None, :, :]
    ).reshape(K_ALL, N_VERT * 3).astype(np.float16)

    w_all = np.zeros((128, W_COLS), np.float16)
    for c, ((js, je), o1, o2, o3) in enumerate(zip(CHUNKS, W1_OFF, W2_OFF, W3_OFF)):
        r0 = 64 * (c % 2)
        for t, j in enumerate(range(js, je)):
            w_all[r0 + t * 9 : r0 + (t + 1) * 9, o1 + t * 18 : o1 + (t + 1) * 18] = w1[j]
            w_all[t * 18 : (t + 1) * 18, o2 + t * 32 : o2 + (t + 1) * 32] = w2[j]
            w_all[t * 32 : (t + 1) * 32, o3 + t * 8 : o3 + (t + 1) * 8] = w3[j]

    bias_all = np.zeros((128, BIAS_COLS), np.float32)
    eye = np.zeros(9, np.float32)
    eye[[0, 4, 8]] = 1.0
    b1_eff = b1 - np.einsum("jio,i->jo", w1, eye)  # fold pose - eye into bias
    for c, (js, je) in enumerate(CHUNKS):
        nj = je - js
        bias_all[0 : 18 * nj, c] = b1_eff[js:je].reshape(-1)
        bias_all[0 : 32 * nj, 6 + c] = b2[js:je].reshape(-1)
    bias_all[0:128, 12] = b3[0:16].reshape(-1)
    bias_all[0:56, 13] = b3[16:23].reshape(-1)

    return pose_t, w_all, bias_all, bfm


def _in_maps(pose, basis, mask, w1, b1, w2, b2, w3, b3):
    pose_t, w_all, bias_all, bfm = _pack_host(
        np.asarray(pose, np.float32),
        np.asarray(basis, np.float32),
        np.asarray(mask, np.float32),
        np.asarray(w1, np.float32),
        np.asarray(b1, np.float32),
        np.asarray(w2, np.float32),
        np.asarray(b2, np.float32),
        np.asarray(w3, np.float32),
        np.asarray(b3, np.float32),
    )
    maps = []
    for i in range(8):
        c0 = i * VC3
        maps.append(
            {
                "pose_t": pose_t,
                "w_all": w_all,
                "bias_all": bias_all,
                "bfm": np.ascontiguousarray(bfm[:, c0 : c0 + VC3]),
            }
        )
    return maps


def kernel(pose, basis, mask, w1, b1, w2, b2, w3, b3):
    from concourse.bass_utils import run_bass_kernel_spmd

    if "nc" not in _CACHED:
        _CACHED["nc"] = _build_nc()
    nc = _CACHED["nc"]

    maps = _in_maps(pose, basis, mask, w1, b1, w2, b2, w3, b3)
    r = run_bass_kernel_spmd(nc, maps, core_ids=list(range(8)))
    out = np.concatenate(
        [m["res"].reshape(BATCH, VC, 3) for m in r.results], axis=1
    )
    return np.ascontiguousarray(out[:, :N_VERT, :])



# revision 5
# speedup vs baseline: 1.0125x; 1.0125x over previous
"""BlendShapes model kernel for 8 Trainium2 NeuronCores.

Computation (reference):
    pose_repr = pose[:, 1:].reshape(B, 23, 9) - eye      # (B, J, 9)
    per-joint MLP 9 -> 18 -> 32 -> 8 (ReLU between)      # coff (B, J, 8)
    basis_full = basis[:, None] * mask[:, :, None, None]  # (V, J, 8, 3)
    res = einsum('bjk,vjkc->bvc', coff, basis_full)       # (B, V, 3)

Mapping (v2):
  - Vertices sharded across 8 cores (V=6890 padded to 8*864=6912; VC3=2592
    output columns per core). Each core runs the full MLP with activations
    transposed ([features, batch]) so coff^T feeds the main matmul's
    stationary operand directly.
  - basis*mask is precomputed on the host as one fp16 [184, VC3] tensor per
    core, scaled by 2^6 (keeps fp16 normals); the matching 2^-6 is folded
    into the L3 bias-add epilogue, so the main matmul's PSUM holds the FINAL
    result and evacuation is a pure f32->f16 copy (no descale pass).
  - Output is stored fp16 (halves HBM write to 5.3 MB/core); the host
    upcasts to f32. Well within the 2e-2 tolerance.
  - The identity subtraction (pose - eye) is folded into the L1 bias.
  - Stores go as ONE whole-row DMA per b-tile ([128 x 5184B] packets) and
    round-robin over THREE DGE rings (sync/scalar/gpsimd). bfm loads on the
    gpsimd ring in parallel with pose/weights at startup.
  - PSUM evacuation alternates DVE / ACT per 1024-col pair so neither
    engine falls behind the tensor engine (which caused the baseline's
    9.4us PSUM stall + 22us store tail).
"""

import numpy as np

N_VERT, N_JOINT, BPJ, BATCH = 6890, 23, 8, 1024
VPAD = 6912  # 8 * 864
VC = VPAD // 8  # 864 vertices per core
VC3 = VC * 3  # 2592
K_ALL = N_JOINT * BPJ  # 184
NB = BATCH // 128  # 8 b-tiles

# Unified joint chunking for the MLP: 4 joints per chunk (3 in the tail).
CHUNKS = [(0, 4), (4, 8), (8, 12), (12, 16), (16, 20), (20, 23)]
NCH = len(CHUNKS)


def _offsets(mpj):
    offs, col = [], 0
    for js, je in CHUNKS:
        offs.append(col)
        col += (je - js) * mpj
    return offs, col


W1_OFF, W1_TOT = _offsets(18)  # 414
W2_OFF, W2_TOT = _offsets(32)  # 736
W3_OFF, W3_TOT = _offsets(8)   # 184
W2_OFF = [W1_TOT + o for o in W2_OFF]
W3_OFF = [W1_TOT + W2_TOT + o for o in W3_OFF]
W_COLS = W1_TOT + W2_TOT + W3_TOT  # 1334

# bias_all columns: [0:6] L1 bias (eye term folded in), [6:12] L2 bias,
# [12] L3 bias (x 2^-6, stacked chunks 0-3, 128 rows), [13] same for
# chunks 4-5 (56 rows).
BIAS_COLS = 14
BSCALE = 64.0   # 2**6, exact; folded out via the L3 epilogue scale
COFF_SCALE = 1.0 / 64.0

# Main matmul N pairs: each pair = one [128, 1024] (2-bank) PSUM tile,
# covering columns [1024p, 1024p + w0 + w1) via two matmuls.
PAIR_W = [(512, 512), (512, 512), (512, 32)]  # covers 2592

_CACHED = {}


def _build_nc():
    import concourse.tile as tile
    from concourse import bacc, mybir
    from contextlib import ExitStack

    dt = mybir.dt
    f32, f16 = dt.float32, dt.float16
    AF = mybir.ActivationFunctionType
    ALU = mybir.AluOpType

    nc = bacc.Bacc(None, target_bir_lowering=False)

    # pose packed as [100, 3072]: chunk c at rows [64*(c%2), +9nj), columns
    # [1024*(c//2), +1024). Row bands at 0/64 because a matmul's moving
    # operand must start at partition 0, 32, or 64.
    pose_t = nc.dram_tensor("pose_t", [100, 3 * BATCH], f16, kind="ExternalInput")
    w_all = nc.dram_tensor("w_all", [128, W_COLS], f16, kind="ExternalInput")
    bias_all = nc.dram_tensor("bias_all", [128, BIAS_COLS], f32, kind="ExternalInput")
    bfm = nc.dram_tensor("bfm", [K_ALL, VC3], f16, kind="ExternalInput")
    res = nc.dram_tensor("res", [BATCH, VC3], f16, kind="ExternalOutput")

    with ExitStack() as ctx:
        tc = ctx.enter_context(tile.TileContext(nc))
        const = ctx.enter_context(tc.tile_pool(name="const", bufs=1))
        work = ctx.enter_context(tc.tile_pool(name="work", bufs=1))
        outp = ctx.enter_context(tc.tile_pool(name="outp", bufs=3))
        pmlp = ctx.enter_context(tc.tile_pool(name="pmlp", bufs=2, space="PSUM"))
        pmain = ctx.enter_context(tc.tile_pool(name="pmain", bufs=3, space="PSUM"))

        # ---- input DMAs. Critical path first: the L1 weight block (scalar
        # ring) and pose half 0 (sync ring) are the only gates for the first
        # matmul; bfm rides the otherwise-idle gpsimd ring in parallel.
        pose_sb = work.tile([100, 3 * BATCH], f16, tag="pose")
        w_sb = const.tile([128, W_COLS], f16, tag="w")
        bias_sb = const.tile([128, BIAS_COLS], f32, tag="bias")
        bfm_a = work.tile([128, VC3], f16, tag="bfm_a")
        bfm_b = work.tile([56, VC3], f16, tag="bfm_b")

        nc.scalar.dma_start(out=w_sb[:, 0:W1_TOT], in_=w_all[:, 0:W1_TOT])
        nc.sync.dma_start(
            out=pose_sb[:, 0 : 2 * BATCH], in_=pose_t[:, 0 : 2 * BATCH]
        )
        nc.gpsimd.dma_start(out=bfm_a[:], in_=bfm[0:128, :])
        nc.scalar.dma_start(out=w_sb[:, W1_TOT:], in_=w_all[:, W1_TOT:])
        nc.sync.dma_start(out=bias_sb[:], in_=bias_all[:, :])
        nc.gpsimd.dma_start(out=bfm_b[:], in_=bfm[128:K_ALL, :])
        nc.sync.dma_start(
            out=pose_sb[:, 2 * BATCH : 3 * BATCH], in_=pose_t[:, 2 * BATCH : 3 * BATCH]
        )

        def pose_ap(c, hs):
            K = 9 * (CHUNKS[c][1] - CHUNKS[c][0])
            r0 = 64 * (c % 2)
            c0 = BATCH * (c // 2)
            return pose_sb[r0 : r0 + K, c0 + hs.start : c0 + hs.stop]

        coffT_a = work.tile([128, BATCH], f16, tag="coffT_a")
        coffT_b = work.tile([56, BATCH], f16, tag="coffT_b")
        h1 = {}
        h2 = {}

        def mlp_half(h):
            hs = slice(h * 512, (h + 1) * 512)
            # L1: 9nj -> 18nj, ReLU(x + b_eff). Chunks 2-4 borrow PSUM from
            # the pmain pool (idle during the MLP phase) so all six L1
            # matmuls can issue without waiting on epilogue recycling; the 3
            # borrowed allocations keep pmain's bufs=3 rotation aligned for
            # the following b-tiles.
            for c, (js, je) in enumerate(CHUNKS):
                nj = je - js
                K, M = 9 * nj, 18 * nj
                off = W1_OFF[c]
                r0 = 64 * (c % 2)
                if c in (2, 3, 4):
                    ps = pmain.tile([M, 512], f32, tag="pmain", name=f"ps1_{c}_{h}")
                else:
                    ps = pmlp.tile([M, 512], f32, tag="psmlp", name=f"ps1_{c}_{h}")
                nc.tensor.matmul(
                    ps[:], lhsT=w_sb[r0 : r0 + K, off : off + M], rhs=pose_ap(c, hs),
                    start=True, stop=True,
                )
                if h == 0:
                    h1[c] = work.tile([M, BATCH], f16, tag=f"h1_{c}", name=f"h1_{c}")
                if c % 2 == 0:
                    nc.scalar.activation(
                        h1[c][:, hs], ps[:], AF.Relu, bias=bias_sb[0:M, c : c + 1]
                    )
                else:
                    nc.vector.tensor_scalar(
                        out=h1[c][:, hs], in0=ps[:],
                        scalar1=bias_sb[0:M, c : c + 1], scalar2=0.0,
                        op0=ALU.add, op1=ALU.max,
                    )
            # L2: 18nj -> 32nj, ReLU; epilogues split ACT/DVE
            for c, (js, je) in enumerate(CHUNKS):
                nj = je - js
                K, M = 18 * nj, 32 * nj
                off = W2_OFF[c]
                ps = pmlp.tile([M, 512], f32, tag="psmlp", name=f"ps2_{c}_{h}")
                nc.tensor.matmul(
                    ps[:], lhsT=w_sb[0:K, off : off + M], rhs=h1[c][:, hs],
                    start=True, stop=True,
                )
                if h == 0:
                    h2[c] = work.tile([M, BATCH], f16, tag=f"h2_{c}", name=f"h2_{c}")
                if c % 2 == 0:
                    nc.scalar.activation(
                        h2[c][:, hs], ps[:], AF.Relu, bias=bias_sb[0:M, 6 + c : 7 + c]
                    )
                else:
                    nc.vector.tensor_scalar(
                        out=h2[c][:, hs], in0=ps[:],
                        scalar1=bias_sb[0:M, 6 + c : 7 + c], scalar2=0.0,
                        op0=ALU.add, op1=ALU.max,
                    )
            # L3: 32nj -> 8nj into stacked PSUM tiles (chunk c at partition
            # 32c); one epilogue each computes (ps + b3) * 2^-6 -> coffT,
            # cancelling bfm's 2^6 so the main matmul emits final values.
            ps3a = pmlp.tile([128, 512], f32, tag="psmlp", name=f"ps3a_{h}")
            ps3b = pmlp.tile([56, 512], f32, tag="psmlp", name=f"ps3b_{h}")
            for c, (js, je) in enumerate(CHUNKS):
                nj = je - js
                K, M = 32 * nj, 8 * nj
                off = W3_OFF[c]
                if c < 4:
                    r0 = 32 * c
                    dst = ps3a[r0 : r0 + M, :]
                else:
                    r0 = 32 * (c - 4)
                    dst = ps3b[r0 : r0 + M, :]
                nc.tensor.matmul(
                    dst, lhsT=w_sb[0:K, off : off + M], rhs=h2[c][:, hs],
                    start=True, stop=True, tile_position=(0, r0),
                )
            nc.vector.tensor_scalar(
                out=coffT_a[:, hs], in0=ps3a[:], scalar1=bias_sb[0:128, 12:13],
                scalar2=COFF_SCALE, op0=ALU.add, op1=ALU.mult,
            )
            nc.vector.tensor_scalar(
                out=coffT_b[:, hs], in0=ps3b[:], scalar1=bias_sb[0:56, 13:14],
                scalar2=COFF_SCALE, op0=ALU.add, op1=ALU.mult,
            )

        STORE_RINGS = [None, None, None]

        def main_btile(bt):
            bsl = slice(bt * 128, (bt + 1) * 128)
            ostrip = outp.tile([128, VC3], f16, tag="ostrip", name=f"ostrip_{bt}")
            ps = [
                pmain.tile([128, 1024], f32, tag="pmain", name=f"ps_{bt}_{p}")
                for p in range(3)
            ]
            # K pass a (coffT_a stationary for all 6 N-slices; N=512 is the
            # ISA max), then pass b accumulating; pair p evacuates + stores
            # right after its b matmuls.
            for p, (w0, w1) in enumerate(PAIR_W):
                c0 = 1024 * p
                nc.tensor.matmul(
                    ps[p][:, 0:w0], lhsT=coffT_a[:, bsl],
                    rhs=bfm_a[:, c0 : c0 + w0], start=True, stop=False,
                )
                nc.tensor.matmul(
                    ps[p][:, 512 : 512 + w1], lhsT=coffT_a[:, bsl],
                    rhs=bfm_a[:, c0 + 512 : c0 + 512 + w1], start=True, stop=False,
                )
            for p, (w0, w1) in enumerate(PAIR_W):
                c0 = 1024 * p
                nc.tensor.matmul(
                    ps[p][:, 0:w0], lhsT=coffT_b[:, bsl],
                    rhs=bfm_b[:, c0 : c0 + w0], start=False, stop=True,
                )
                nc.tensor.matmul(
                    ps[p][:, 512 : 512 + w1], lhsT=coffT_b[:, bsl],
                    rhs=bfm_b[:, c0 + 512 : c0 + 512 + w1], start=False, stop=True,
                )
                # Evacuate the pair as a pure f32->f16 copy, alternating
                # DVE / ACT so each engine carries ~1.5 pairs per b-tile.
                wp = w0 + w1
                osl = slice(c0, c0 + wp)
                if (bt + p) % 2 == 0:
                    nc.vector.tensor_copy(out=ostrip[:, osl], in_=ps[p][:, 0:wp])
                else:
                    nc.scalar.copy(out=ostrip[:, osl], in_=ps[p][:, 0:wp])
                if bt == NB - 1:
                    # Final b-tile: store per pair so the tail is just the
                    # last pair's evac + a ~0.9us store.
                    ring = [nc.sync, nc.scalar, nc.gpsimd][p]
                    ring.dma_start(out=res[bsl, osl], in_=ostrip[:, osl])
            if bt < NB - 1:
                # One whole-row store per b-tile: 128 packets x 5184B,
                # round-robin across the three DGE rings.
                ring = [nc.sync, nc.scalar, nc.gpsimd][bt % 3]
                ring.dma_start(out=res[bsl, :], in_=ostrip[:, :])

        mlp_half(0)
        for bt in range(4):
            main_btile(bt)
        mlp_half(1)
        for bt in range(4, NB):
            main_btile(bt)

    nc.finalize()
    return nc


def _pack_host(pose, basis, mask, w1, b1, w2, b2, w3, b3):
    pose_flat = pose[:, 1:].reshape(BATCH, 207).T.astype(np.float16)  # [207, B]
    pose_t = np.zeros((100, 3 * BATCH), np.float16)
    for c, (js, je) in enumerate(CHUNKS):
        K = 9 * (je - js)
        r0, c0 = 64 * (c % 2), BATCH * (c // 2)
        pose_t[r0 : r0 + K, c0 : c0 + BATCH] = pose_flat[9 * js : 9 * js + K]

    # bfm[8j+k, (v, c)] = mask[v, j] * basis[v, k, c] * 2^6, fp16
    basis_t = basis.transpose(1, 0, 2).reshape(BPJ, N_VERT * 3)  # [k, v3]
    mask3 = np.repeat(mask.T, 3, axis=1)  # [j, v3]
    bfm = np.zeros((K_ALL, VPAD * 3), np.float16)
    bfm[:, : N_VERT * 3] = (
        mask3[:, None, :] * (basis_t * BSCALE)[None, :, :]
    ).reshape(K_ALL, N_VERT * 3).astype(np.float16)

    w_all = np.zeros((128, W_COLS), np.float16)
    for c, ((js, je), o1, o2, o3) in enumerate(zip(CHUNKS, W1_OFF, W2_OFF, W3_OFF)):
        r0 = 64 * (c % 2)
        for t, j in enumerate(range(js, je)):
            w_all[r0 + t * 9 : r0 + (t + 1) * 9, o1 + t * 18 : o1 + (t + 1) * 18] = w1[j]
            w_all[t * 18 : (t + 1) * 18, o2 + t * 32 : o2 + (t + 1) * 32] = w2[j]
            w_all[t * 32 : (t + 1) * 32, o3 + t * 8 : o3 + (t + 1) * 8] = w3[j]

    bias_all = np.zeros((128, BIAS_COLS), np.float32)
    eye = np.zeros(9, np.float32)
    eye[[0, 4, 8]] = 1.0
    b1_eff = b1 - np.einsum("jio,i->jo", w1, eye)  # fold pose - eye into bias
    for c, (js, je) in enumerate(CHUNKS):
        nj = je - js
        bias_all[0 : 18 * nj, c] = b1_eff[js:je].reshape(-1)
        bias_all[0 : 32 * nj, 6 + c] = b2[js:je].reshape(-1)
    # L3 bias, unscaled: both epilogues compute (psum + b3) * 2^-6 on DVE.
    bias_all[0:128, 12] = b3[0:16].reshape(-1)
    bias_all[0:56, 13] = b3[16:23].reshape(-1)

    return pose_t, w_all, bias_all, bfm


def _in_maps(pose, basis, mask, w1, b1, w2, b2, w3, b3):
    pose_t, w_all, bias_all, bfm = _pack_host(
        np.asarray(pose, np.float32),
        np.asarray(basis, np.float32),
        np.asarray(mask, np.float32),
        np.asarray(w1, np.float32),
        np.asarray(b1, np.float32),
        np.asarray(w2, np.float32),
        np.asarray(b2, np.float32),
        np.asarray(w3, np.float32),
        np.asarray(b3, np.float32),
    )
    maps = []
    for i in range(8):
        c0 = i * VC3
        maps.append(
            {
                "pose_t": pose_t,
                "w_all": w_all,
                "bias_all": bias_all,
                "bfm": np.ascontiguousarray(bfm[:, c0 : c0 + VC3]),
            }
        )
    return maps


def kernel(pose, basis, mask, w1, b1, w2, b2, w3, b3):
    from concourse.bass_utils import run_bass_kernel_spmd

    if "nc" not in _CACHED:
        _CACHED["nc"] = _build_nc()
    nc = _CACHED["nc"]

    maps = _in_maps(pose, basis, mask, w1, b1, w2, b2, w3, b3)
    r = run_bass_kernel_spmd(nc, maps, core_ids=list(range(8)))
    out = np.concatenate(
        [m["res"].reshape(BATCH, VC, 3) for m in r.results], axis=1
    )
    return np.ascontiguousarray(out[:, :N_VERT, :].astype(np.float32))


# revision 8
# speedup vs baseline: 1.0612x; 1.0481x over previous
"""BlendShapes model kernel for 8 Trainium2 NeuronCores.

Computation (reference):
    pose_repr = pose[:, 1:].reshape(B, 23, 9) - eye      # (B, J, 9)
    per-joint MLP 9 -> 18 -> 32 -> 8 (ReLU between)      # coff (B, J, 8)
    basis_full = basis[:, None] * mask[:, :, None, None]  # (V, J, 8, 3)
    res = einsum('bjk,vjkc->bvc', coff, basis_full)       # (B, V, 3)

Mapping (v2):
  - Vertices sharded across 8 cores (V=6890 padded to 8*864=6912; VC3=2592
    output columns per core). Each core runs the full MLP with activations
    transposed ([features, batch]) so coff^T feeds the main matmul's
    stationary operand directly.
  - basis*mask is precomputed on the host as one fp16 [184, VC3] tensor per
    core, scaled by 2^6 (keeps fp16 normals); the matching 2^-6 is folded
    into the L3 bias-add epilogue, so the main matmul's PSUM holds the FINAL
    result and evacuation is a pure f32->f16 copy (no descale pass).
  - Output is stored fp16 (halves HBM write to 5.3 MB/core); the host
    upcasts to f32. Well within the 2e-2 tolerance.
  - The identity subtraction (pose - eye) is folded into the L1 bias.
  - Stores go as ONE whole-row DMA per b-tile ([128 x 5184B] packets) and
    round-robin over THREE DGE rings (sync/scalar/gpsimd). bfm loads on the
    gpsimd ring in parallel with pose/weights at startup.
  - PSUM evacuation alternates DVE / ACT per 1024-col pair so neither
    engine falls behind the tensor engine (which caused the baseline's
    9.4us PSUM stall + 22us store tail).
"""

import numpy as np

N_VERT, N_JOINT, BPJ, BATCH = 6890, 23, 8, 1024
VPAD = 6912  # 8 * 864
VC = VPAD // 8  # 864 vertices per core
VC3 = VC * 3  # 2592
K_ALL = N_JOINT * BPJ  # 184
NB = BATCH // 128  # 8 b-tiles

# Unified joint chunking for the MLP: 4 joints per chunk (3 in the tail).
CHUNKS = [(0, 4), (4, 8), (8, 12), (12, 16), (16, 20), (20, 23)]
NCH = len(CHUNKS)


def _offsets(mpj):
    offs, col = [], 0
    for js, je in CHUNKS:
        offs.append(col)
        col += (je - js) * mpj
    return offs, col


W1_OFF, W1_TOT = _offsets(18)  # 414
W2_OFF, W2_TOT = _offsets(32)  # 736
W3_OFF, W3_TOT = _offsets(8)   # 184
W2_OFF = [W1_TOT + o for o in W2_OFF]
W3_OFF = [W1_TOT + W2_TOT + o for o in W3_OFF]
W_COLS = W1_TOT + W2_TOT + W3_TOT  # 1334

# bias_all columns: [0:6] L1 bias (eye term folded in), [6:12] L2 bias,
# [12] L3 bias (x 2^-6, stacked chunks 0-3, 128 rows), [13] same for
# chunks 4-5 (56 rows).
BIAS_COLS = 14
BSCALE = 64.0   # 2**6, exact; folded out via the L3 epilogue scale
COFF_SCALE = 1.0 / 64.0

# Main matmul N pairs: each pair = one [128, 1024] (2-bank) PSUM tile,
# covering columns [1024p, 1024p + w0 + w1) via two matmuls.
PAIR_W = [(512, 512), (512, 512), (512, 32)]  # covers 2592

_CACHED = {}


def _build_nc():
    import concourse.tile as tile
    from concourse import bacc, mybir
    from contextlib import ExitStack

    dt = mybir.dt
    f32, f16 = dt.float32, dt.float16
    AF = mybir.ActivationFunctionType
    ALU = mybir.AluOpType

    nc = bacc.Bacc(None, target_bir_lowering=False)

    # pose packed as [100, 3072]: chunk c at rows [64*(c%2), +9nj), columns
    # [1024*(c//2), +1024). Row bands at 0/64 because a matmul's moving
    # operand must start at partition 0, 32, or 64.
    pose_t = nc.dram_tensor("pose_t", [100, 3 * BATCH], f16, kind="ExternalInput")
    w_all = nc.dram_tensor("w_all", [128, W_COLS], f16, kind="ExternalInput")
    bias_all = nc.dram_tensor("bias_all", [128, BIAS_COLS], f32, kind="ExternalInput")
    bfm = nc.dram_tensor("bfm", [K_ALL, VC3], f16, kind="ExternalInput")
    res = nc.dram_tensor("res", [BATCH, VC3], f16, kind="ExternalOutput")

    with ExitStack() as ctx:
        tc = ctx.enter_context(tile.TileContext(nc))
        const = ctx.enter_context(tc.tile_pool(name="const", bufs=1))
        work = ctx.enter_context(tc.tile_pool(name="work", bufs=1))
        outp = ctx.enter_context(tc.tile_pool(name="outp", bufs=3))
        pmlp = ctx.enter_context(tc.tile_pool(name="pmlp", bufs=2, space="PSUM"))
        pmain = ctx.enter_context(tc.tile_pool(name="pmain", bufs=3, space="PSUM"))

        # ---- input DMAs. Critical path first: the L1 weight block (scalar
        # ring) and pose half 0 (sync ring) are the only gates for the first
        # matmul; bfm rides the otherwise-idle gpsimd ring in parallel.
        pose_sb = work.tile([100, 3 * BATCH], f16, tag="pose")
        w_sb = const.tile([128, W_COLS], f16, tag="w")
        bias_sb = const.tile([128, BIAS_COLS], f32, tag="bias")
        bfm_a = work.tile([128, VC3], f16, tag="bfm_a")
        bfm_b = work.tile([56, VC3], f16, tag="bfm_b")

        # Tiny first transfers so L1 chunk 0 can issue ~1.5us after the DGE
        # rings open: chunk 0's weight block and its pose rows only.
        nc.scalar.dma_start(out=w_sb[0:36, 0:72], in_=w_all[0:36, 0:72])
        nc.sync.dma_start(out=pose_sb[0:36, 0:BATCH], in_=pose_t[0:36, 0:BATCH])
        nc.scalar.dma_start(out=w_sb[64:100, 72:144], in_=w_all[64:100, 72:144])
        nc.sync.dma_start(
            out=pose_sb[64:100, 0:BATCH], in_=pose_t[64:100, 0:BATCH]
        )
        nc.gpsimd.dma_start(out=bfm_a[:], in_=bfm[0:128, :])
        nc.scalar.dma_start(out=w_sb[:, 144:W1_TOT], in_=w_all[:, 144:W1_TOT])
        nc.sync.dma_start(
            out=pose_sb[:, BATCH : 2 * BATCH], in_=pose_t[:, BATCH : 2 * BATCH]
        )
        nc.scalar.dma_start(out=bias_sb[:], in_=bias_all[:, :])
        nc.sync.dma_start(
            out=pose_sb[:, 2 * BATCH : 3 * BATCH], in_=pose_t[:, 2 * BATCH : 3 * BATCH]
        )
        nc.scalar.dma_start(out=w_sb[:, W1_TOT:], in_=w_all[:, W1_TOT:])
        nc.gpsimd.dma_start(out=bfm_b[:], in_=bfm[128:K_ALL, :])

        def pose_ap(c, hs):
            K = 9 * (CHUNKS[c][1] - CHUNKS[c][0])
            r0 = 64 * (c % 2)
            c0 = BATCH * (c // 2)
            return pose_sb[r0 : r0 + K, c0 + hs.start : c0 + hs.stop]

        coffT_a = work.tile([128, BATCH], f16, tag="coffT_a")
        coffT_b = work.tile([56, BATCH], f16, tag="coffT_b")
        h1 = {}
        h2 = {}

        def mlp_half(h):
            hs = slice(h * 512, (h + 1) * 512)
            # L1: 9nj -> 18nj, ReLU(x + b_eff). Chunks 2-4 borrow PSUM from
            # the pmain pool (idle during the MLP phase) so all six L1
            # matmuls can issue without waiting on epilogue recycling; the 3
            # borrowed allocations keep pmain's bufs=3 rotation aligned for
            # the following b-tiles.
            for c, (js, je) in enumerate(CHUNKS):
                nj = je - js
                K, M = 9 * nj, 18 * nj
                off = W1_OFF[c]
                r0 = 64 * (c % 2)
                if c in (2, 3, 4):
                    ps = pmain.tile([M, 512], f32, tag="pmain", name=f"ps1_{c}_{h}")
                else:
                    ps = pmlp.tile([M, 512], f32, tag="psmlp", name=f"ps1_{c}_{h}")
                nc.tensor.matmul(
                    ps[:], lhsT=w_sb[r0 : r0 + K, off : off + M], rhs=pose_ap(c, hs),
                    start=True, stop=True,
                )
                if h == 0:
                    h1[c] = work.tile([M, BATCH], f16, tag=f"h1_{c}", name=f"h1_{c}")
                if c % 2 == 0:
                    nc.scalar.activation(
                        h1[c][:, hs], ps[:], AF.Relu, bias=bias_sb[0:M, c : c + 1]
                    )
                else:
                    nc.vector.tensor_scalar(
                        out=h1[c][:, hs], in0=ps[:],
                        scalar1=bias_sb[0:M, c : c + 1], scalar2=0.0,
                        op0=ALU.add, op1=ALU.max,
                    )
            # L2: 18nj -> 32nj, ReLU; epilogues split ACT/DVE
            for c, (js, je) in enumerate(CHUNKS):
                nj = je - js
                K, M = 18 * nj, 32 * nj
                off = W2_OFF[c]
                ps = pmlp.tile([M, 512], f32, tag="psmlp", name=f"ps2_{c}_{h}")
                nc.tensor.matmul(
                    ps[:], lhsT=w_sb[0:K, off : off + M], rhs=h1[c][:, hs],
                    start=True, stop=True,
                )
                if h == 0:
                    h2[c] = work.tile([M, BATCH], f16, tag=f"h2_{c}", name=f"h2_{c}")
                if c % 2 == 0:
                    nc.scalar.activation(
                        h2[c][:, hs], ps[:], AF.Relu, bias=bias_sb[0:M, 6 + c : 7 + c]
                    )
                else:
                    nc.vector.tensor_scalar(
                        out=h2[c][:, hs], in0=ps[:],
                        scalar1=bias_sb[0:M, 6 + c : 7 + c], scalar2=0.0,
                        op0=ALU.add, op1=ALU.max,
                    )
            # L3: 32nj -> 8nj into stacked PSUM tiles (chunk c at partition
            # 32c); one epilogue each computes (ps + b3) * 2^-6 -> coffT,
            # cancelling bfm's 2^6 so the main matmul emits final values.
            ps3a = pmlp.tile([128, 512], f32, tag="psmlp", name=f"ps3a_{h}")
            ps3b = pmlp.tile([56, 512], f32, tag="psmlp", name=f"ps3b_{h}")
            for c, (js, je) in enumerate(CHUNKS):
                nj = je - js
                K, M = 32 * nj, 8 * nj
                off = W3_OFF[c]
                if c < 4:
                    r0 = 32 * c
                    dst = ps3a[r0 : r0 + M, :]
                else:
                    r0 = 32 * (c - 4)
                    dst = ps3b[r0 : r0 + M, :]
                nc.tensor.matmul(
                    dst, lhsT=w_sb[0:K, off : off + M], rhs=h2[c][:, hs],
                    start=True, stop=True, tile_position=(0, r0),
                )
            nc.vector.tensor_scalar(
                out=coffT_a[:, hs], in0=ps3a[:], scalar1=bias_sb[0:128, 12:13],
                scalar2=COFF_SCALE, op0=ALU.add, op1=ALU.mult,
            )
            nc.vector.tensor_scalar(
                out=coffT_b[:, hs], in0=ps3b[:], scalar1=bias_sb[0:56, 13:14],
                scalar2=COFF_SCALE, op0=ALU.add, op1=ALU.mult,
            )

        STORE_RINGS = [None, None, None]

        def main_btile(bt):
            bsl = slice(bt * 128, (bt + 1) * 128)
            ostrip = outp.tile([128, VC3], f16, tag="ostrip", name=f"ostrip_{bt}")
            ps = [
                pmain.tile([128, 1024], f32, tag="pmain", name=f"ps_{bt}_{p}")
                for p in range(3)
            ]
            # K pass a (coffT_a stationary for all 6 N-slices; N=512 is the
            # ISA max), then pass b accumulating; pair p evacuates + stores
            # right after its b matmuls.
            for p, (w0, w1) in enumerate(PAIR_W):
                c0 = 1024 * p
                nc.tensor.matmul(
                    ps[p][:, 0:w0], lhsT=coffT_a[:, bsl],
                    rhs=bfm_a[:, c0 : c0 + w0], start=True, stop=False,
                )
                nc.tensor.matmul(
                    ps[p][:, 512 : 512 + w1], lhsT=coffT_a[:, bsl],
                    rhs=bfm_a[:, c0 + 512 : c0 + 512 + w1], start=True, stop=False,
                )
            for p, (w0, w1) in enumerate(PAIR_W):
                c0 = 1024 * p
                nc.tensor.matmul(
                    ps[p][:, 0:w0], lhsT=coffT_b[:, bsl],
                    rhs=bfm_b[:, c0 : c0 + w0], start=False, stop=True,
                )
                nc.tensor.matmul(
                    ps[p][:, 512 : 512 + w1], lhsT=coffT_b[:, bsl],
                    rhs=bfm_b[:, c0 + 512 : c0 + 512 + w1], start=False, stop=True,
                )
                # Evacuate the pair as a pure f32->f16 copy, alternating
                # DVE / ACT so each engine carries ~1.5 pairs per b-tile.
                wp = w0 + w1
                osl = slice(c0, c0 + wp)
                if (bt + p) % 2 == 0:
                    nc.vector.tensor_scalar(
                        out=ostrip[:, osl], in0=ps[p][:, 0:wp],
                        scalar1=0.0, scalar2=None, op0=ALU.add,
                    )
                else:
                    nc.scalar.copy(out=ostrip[:, osl], in_=ps[p][:, 0:wp])
                if bt == NB - 1:
                    # Final b-tile: store per pair so the tail is just the
                    # last pair's evac + a ~0.9us store.
                    ring = [nc.sync, nc.scalar, nc.gpsimd][p]
                    ring.dma_start(out=res[bsl, osl], in_=ostrip[:, osl])
            if bt < NB - 1:
                # One whole-row store per b-tile: 128 packets x 5184B,
                # round-robin across the three DGE rings.
                ring = [nc.sync, nc.scalar, nc.gpsimd][bt % 3]
                ring.dma_start(out=res[bsl, :], in_=ostrip[:, :])

        # Both MLP halves up front, then 8 uninterrupted main b-tiles: the
        # PE p-state ramps with sustained gapless execution, so one long
        # matmul stream beats interleaving the MLP mid-kernel.
        mlp_half(0)
        mlp_half(1)
        for bt in range(NB):
            main_btile(bt)

    nc.finalize()
    return nc


def _pack_host(pose, basis, mask, w1, b1, w2, b2, w3, b3):
    pose_flat = pose[:, 1:].reshape(BATCH, 207).T.astype(np.float16)  # [207, B]
    pose_t = np.zeros((100, 3 * BATCH), np.float16)
    for c, (js, je) in enumerate(CHUNKS):
        K = 9 * (je - js)
        r0, c0 = 64 * (c % 2), BATCH * (c // 2)
        pose_t[r0 : r0 + K, c0 : c0 + BATCH] = pose_flat[9 * js : 9 * js + K]

    # bfm[8j+k, (v, c)] = mask[v, j] * basis[v, k, c] * 2^6, fp16
    basis_t = basis.transpose(1, 0, 2).reshape(BPJ, N_VERT * 3)  # [k, v3]
    mask3 = np.repeat(mask.T, 3, axis=1)  # [j, v3]
    bfm = np.zeros((K_ALL, VPAD * 3), np.float16)
    bfm[:, : N_VERT * 3] = (
        mask3[:, None, :] * (basis_t * BSCALE)[None, :, :]
    ).reshape(K_ALL, N_VERT * 3).astype(np.float16)

    w_all = np.zeros((128, W_COLS), np.float16)
    for c, ((js, je), o1, o2, o3) in enumerate(zip(CHUNKS, W1_OFF, W2_OFF, W3_OFF)):
        r0 = 64 * (c % 2)
        for t, j in enumerate(range(js, je)):
            w_all[r0 + t * 9 : r0 + (t + 1) * 9, o1 + t * 18 : o1 + (t + 1) * 18] = w1[j]
            w_all[t * 18 : (t + 1) * 18, o2 + t * 32 : o2 + (t + 1) * 32] = w2[j]
            w_all[t * 32 : (t + 1) * 32, o3 + t * 8 : o3 + (t + 1) * 8] = w3[j]

    bias_all = np.zeros((128, BIAS_COLS), np.float32)
    eye = np.zeros(9, np.float32)
    eye[[0, 4, 8]] = 1.0
    b1_eff = b1 - np.einsum("jio,i->jo", w1, eye)  # fold pose - eye into bias
    for c, (js, je) in enumerate(CHUNKS):
        nj = je - js
        bias_all[0 : 18 * nj, c] = b1_eff[js:je].reshape(-1)
        bias_all[0 : 32 * nj, 6 + c] = b2[js:je].reshape(-1)
    # L3 bias, unscaled: both epilogues compute (psum + b3) * 2^-6 on DVE.
    bias_all[0:128, 12] = b3[0:16].reshape(-1)
    bias_all[0:56, 13] = b3[16:23].reshape(-1)

    return pose_t, w_all, bias_all, bfm


def _in_maps(pose, basis, mask, w1, b1, w2, b2, w3, b3):
    pose_t, w_all, bias_all, bfm = _pack_host(
        np.asarray(pose, np.float32),
        np.asarray(basis, np.float32),
        np.asarray(mask, np.float32),
        np.asarray(w1, np.float32),
        np.asarray(b1, np.float32),
        np.asarray(w2, np.float32),
        np.asarray(b2, np.float32),
        np.asarray(w3, np.float32),
        np.asarray(b3, np.float32),
    )
    maps = []
    for i in range(8):
        c0 = i * VC3
        maps.append(
            {
                "pose_t": pose_t,
                "w_all": w_all,
                "bias_all": bias_all,
                "bfm": np.ascontiguousarray(bfm[:, c0 : c0 + VC3]),
            }
        )
    return maps


def kernel(pose, basis, mask, w1, b1, w2, b2, w3, b3):
    from concourse.bass_utils import run_bass_kernel_spmd

    if "nc" not in _CACHED:
        _CACHED["nc"] = _build_nc()
    nc = _CACHED["nc"]

    maps = _in_maps(pose, basis, mask, w1, b1, w2, b2, w3, b3)
    r = run_bass_kernel_spmd(nc, maps, core_ids=list(range(8)))
    out = np.concatenate(
        [m["res"].reshape(BATCH, VC, 3) for m in r.results], axis=1
    )
    return np.ascontiguousarray(out[:, :N_VERT, :].astype(np.float32))
